# revision 27
# baseline (speedup 1.0000x reference)
"""Trainium2 Bass kernel for nn_CrossAttention (cross-attention + post-softmax
masking + residual + LayerNorm), data-parallel over batch across 8 NeuronCores.

Hardcoded problem shapes:
  B=16, N=256 (drug seq), T=1024 (target seq), DRUG_DIM=768, TGT_DIM=2560,
  H=8 heads, DH=96, INNER=768.

Sharding: batch 16 -> 8 cores x 2 local batches. No collectives.

Per-core dataflow (local batch b in {0,1}):
  targetT [2560,1024] and drugT [768,256] loaded via xbar DMA-transpose (bf16)
  QT_h[dh,n]  = Wq_h^T drugT       (per-head, psum fp32 -> sbuf bf16)
  KT_h[dh,t]  = Wk_h^T targetT
  V_tc[t,i]   = targetT_tc^T Wv    (per t-chunk of 128)
  S[n,t]      = QT_h^T KT_h        (psum fp32, per (h, n-chunk of 128))
  E           = exp(S*scale)       (ACT, accum_out = row sums; no max-subtract:
                                    |S*scale| <= ~3 for this data distribution)
  attn        = E * (dmask/rowsum)[per-row] * pmask[per-col]   (post-softmax
                                    masking, no renorm; in-place fp32)
  attn -> DRAM (fp32), attn -> bf16 (ACT cast) -> xbar transpose -> pT[t,n]
  OT_h[dh,n]  = V_h^T pT           (accumulated over t-chunks)
  fin[n,d]    = sum_h OT_h^T Wo_h  (psum fp32)
  out         = LayerNorm(fin + bo + drug) * gamma + beta -> DRAM (fp32)
"""

import numpy as np
import ml_dtypes

B, N, T = 16, 256, 1024
D, TD, H, DH = 768, 2560, 8, 96
INNER = H * DH
EPS = 1e-5
SCALE = DH ** -0.5

NCORES = 8
BL = B // NCORES          # local batches per core
DCH = TD // 128           # 20 contraction chunks for k/v projections
ICH = D // 128            # 6 contraction chunks for q projection
TCH = T // 128            # 8 t-chunks
NCH = N // 128            # 2 n-chunks

BF16 = "bfloat16"
F32 = "float32"

_compiled = {}


def build_bass():
    """Build and compile the per-core Bass program (same program on all 8 cores)."""
    import concourse.bacc as bacc
    import concourse.tile as tile
    import concourse.bass as bass
    from concourse import mybir
    from concourse.bass import ts, ds

    DT = mybir.dt.bfloat16
    FP = mybir.dt.float32
    AF = mybir.ActivationFunctionType
    OP = mybir.AluOpType

    nc = bacc.Bacc("TRN2", target_bir_lowering=False, debug=False,
                   num_devices=NCORES)

    # ---- DRAM I/O ----
    drug_bf = nc.dram_tensor("drug_bf", [BL, N, D], DT, kind="ExternalInput").ap()
    drug_f = nc.dram_tensor("drug_f", [BL, N, D], FP, kind="ExternalInput").ap()
    target_bf = nc.dram_tensor("target_bf", [BL, T, TD], DT, kind="ExternalInput").ap()
    wq_e = nc.dram_tensor("wq", [D, INNER], DT, kind="ExternalInput").ap()
    wk_e = nc.dram_tensor("wk", [TD, INNER], DT, kind="ExternalInput").ap()
    wv_e = nc.dram_tensor("wv", [TD, INNER], DT, kind="ExternalInput").ap()
    wo_e = nc.dram_tensor("wo", [INNER, D], DT, kind="ExternalInput").ap()
    dmask_e = nc.dram_tensor("dmask", [BL, 128, NCH], FP, kind="ExternalInput").ap()
    pmask_e = nc.dram_tensor("pmask", [BL, T], FP, kind="ExternalInput").ap()
    bo_e = nc.dram_tensor("bo", [D], DT, kind="ExternalInput").ap()
    gamma_e = nc.dram_tensor("gamma", [D], DT, kind="ExternalInput").ap()
    beta_e = nc.dram_tensor("beta", [D], DT, kind="ExternalInput").ap()

    attn_o = nc.dram_tensor("attn_o", [BL, H, N, T], FP, kind="ExternalOutput").ap()
    out_o = nc.dram_tensor("out_o", [BL, N, D], FP, kind="ExternalOutput").ap()

    def bcast(ap, p=128):
        return bass.AP(tensor=ap.tensor, offset=ap.offset,
                       ap=[[0, p]] + [list(x) for x in ap.ap])

    with tile.TileContext(nc) as tc:
        with (
            tc.tile_pool(name="singles", bufs=1) as singles,
            tc.tile_pool(name="ttp", bufs=3) as ttp,
            tc.tile_pool(name="proj", bufs=9) as projp,
            tc.tile_pool(name="ktpool", bufs=10) as ktp,
            tc.tile_pool(name="vtpool", bufs=9) as vtp,
            tc.tile_pool(name="small", bufs=2) as smallp,
            tc.tile_pool(name="small1", bufs=1) as small1p,
            tc.tile_pool(name="attnb", bufs=2) as attnbp,
            tc.tile_pool(name="attnb4", bufs=3) as attnb4p,
            tc.tile_pool(name="attnb3", bufs=3) as attnb3p,
            tc.tile_pool(name="attnbE", bufs=3) as attnbEp,
            tc.tile_pool(name="otp", bufs=18) as otp,
            tc.tile_pool(name="stats", bufs=8) as statsp,
            tc.tile_pool(name="ps_proj", bufs=1, space="PSUM") as ps_proj,
            tc.tile_pool(name="ps_attn", bufs=4, space="PSUM") as ps_attn,
            tc.tile_pool(name="ps_ktq", bufs=2, space="PSUM") as ps_ktq,
        ):
            # ---- resident weights / constants ----
            wq_sb = singles.tile([128, ICH, INNER], DT, tag="wq")
            nc.scalar.dma_start(out=wq_sb, in_=wq_e.rearrange("(c p) n -> p c n", p=128))
            wk_sb = singles.tile([128, DCH, INNER], DT, tag="wk")
            wk_r = wk_e.rearrange("(c p) n -> p c n", p=128)
            nc.scalar.dma_start(out=wk_sb[:, 0:10, :], in_=wk_r[:, 0:10, :])
            nc.sync.dma_start(out=wk_sb[:, 10:DCH, :], in_=wk_r[:, 10:DCH, :])
            wv_sb = singles.tile([128, DCH, INNER], DT, tag="wv")
            wo_sb = singles.tile([96, H, D], DT, tag="wo")
            gamma_b = singles.tile([128, D], DT, tag="gamma")
            beta_b = singles.tile([128, D], DT, tag="beta")
            bo_b = singles.tile([128, D], DT, tag="bo")
            eps_t = singles.tile([128, 1], FP, tag="eps")
            nc.vector.memset(eps_t, EPS)

            def load_wv():
                wv_r = wv_e.rearrange("(c p) n -> p c n", p=128)
                nc.scalar.dma_start(out=wv_sb[:, 0:10, :], in_=wv_r[:, 0:10, :])
                nc.sync.dma_start(out=wv_sb[:, 10:DCH, :], in_=wv_r[:, 10:DCH, :])

            def load_tail_weights():
                nc.scalar.dma_start(out=wo_sb,
                                    in_=wo_e.rearrange("(h p) n -> p h n", p=96))
                nc.scalar.dma_start(out=gamma_b, in_=bcast(gamma_e))
                nc.scalar.dma_start(out=beta_b, in_=bcast(beta_e))
                nc.scalar.dma_start(out=bo_b, in_=bcast(bo_e))

            PVLAG = 3  # PV consumption lags softmax emission by this many units

            # Per-batch state built by the projection work items
            state = [dict(qt=[None] * H, kt=None, vt=[None] * TCH, ot={},
                          drugT=None, drug_x=None, pmask=None, dmask=None)
                     for _ in range(BL)]

            def proj_items(b):
                """Work items (closures) for batch b's loads + projections.

                Emitted either directly (first batch) or interleaved into the
                previous batch's attention to keep the in-order PE stream fed.
                PSUM->SBUF cast copies ride on ACT so the DVE stream (softmax)
                never blocks on PE projection progress.
                """
                st = state[b]

                def loads():
                    st["drugT"] = small1p.tile([128, ICH, N], DT, tag="drugT",
                                               name=f"drugT{b}")
                    nc.sync.dma_start_transpose(out=st["drugT"], in_=drug_bf[b])
                    st["drug_x"] = small1p.tile([128, NCH, D], FP, tag="drugx",
                                                name=f"drugx{b}")
                    nc.sync.dma_start(
                        out=st["drug_x"],
                        in_=drug_f[b].rearrange("(c p) d -> p c d", p=128))
                    st["pmask"] = small1p.tile([128, T], FP, tag="pmask",
                                               name=f"pmask{b}")
                    nc.sync.dma_start(out=st["pmask"], in_=bcast(pmask_e[b]))
                    st["dmask"] = smallp.tile([128, NCH], FP, tag="dmask",
                                              name=f"dmask{b}")
                    nc.sync.dma_start(out=st["dmask"], in_=dmask_e[b])
                    st["kt"] = [ktp.tile([96, T], DT, tag="kt",
                                         name=f"kt{b}_{h}") for h in range(H)]
                yield loads

                def qt_item(h):
                    def f():
                        ps = ps_ktq.tile([96, N], FP, tag="ktq", name=f"qtps{b}{h}")
                        for dc in range(ICH):
                            nc.tensor.matmul(ps,
                                             lhsT=wq_sb[:, dc, ds(DH * h, DH)],
                                             rhs=st["drugT"][:, dc, :],
                                             start=(dc == 0), stop=(dc == ICH - 1))
                        t_ = projp.tile([96, N], DT, tag="qt", name=f"qt{b}{h}")
                        nc.scalar.activation(out=t_, in_=ps, func=AF.Copy)
                        st["qt"][h] = t_
                    return f

                tts = {}

                def tt_item(tq):
                    def f():
                        tt = ttp.tile([128, DCH, 256], DT, tag="tt",
                                      name=f"tt{b}{tq}")
                        nc.sync.dma_start_transpose(
                            out=tt, in_=target_bf[b, ts(tq, 256), :])
                        tts[tq] = tt
                    return f

                def kt_item(h, tq):
                    def f():
                        ps = ps_ktq.tile([96, 256], FP, tag="ktq",
                                          name=f"ktps{b}{h}{tq}")
                        for dc in range(DCH):
                            nc.tensor.matmul(ps,
                                             lhsT=wk_sb[:, dc, ds(DH * h, DH)],
                                             rhs=tts[tq][:, dc, :],
                                             start=(dc == 0), stop=(dc == DCH - 1))
                        nc.scalar.activation(out=st["kt"][h][:, ts(tq, 256)],
                                             in_=ps, func=AF.Copy)
                    return f

                def v_item(tq, lc):
                    def f():
                        tc_i = tq * 2 + lc
                        t_ = vtp.tile([128, INNER], DT, tag="vt",
                                      name=f"vt{b}{tc_i}")
                        ps = ps_proj.tile([128, INNER], FP, tag="proj",
                                          name=f"vps{b}{tc_i}")
                        for n0, nn in ((0, 512), (512, 256)):
                            for dc in range(DCH):
                                nc.tensor.matmul(ps[:, ds(n0, nn)],
                                                 lhsT=tts[tq][:, dc, ts(lc, 128)],
                                                 rhs=wv_sb[:, dc, ds(n0, nn)],
                                                 start=(dc == 0),
                                                 stop=(dc == DCH - 1))
                        nc.scalar.activation(out=t_, in_=ps, func=AF.Copy)
                        st["vt"][tc_i] = t_
                    return f

                yield tt_item(0)
                for h in range(H):
                    yield qt_item(h)
                for tq in range(4):
                    if tq + 1 < 4:
                        yield tt_item(tq + 1)
                    if b == 0 and tq == 0:
                        yield load_wv
                    for h in range(H):
                        yield kt_item(h, tq)
                    if b == 0 and tq == 1:
                        yield load_tail_weights
                    yield v_item(tq, 0)
                    yield v_item(tq, 1)

            def softmax_unit(b, h, nc2, alt=False):
                st = state[b]
                ses = []
                spool = ps_ktq if alt else ps_attn
                stag = "ktq" if alt else "s"
                e_t = attnbEp.tile([128, T], FP, tag="E", name=f"E{b}{h}{nc2}")
                for half in range(2):
                    s_ps = spool.tile([128, 512], FP, tag=stag,
                                      name=f"sps{b}{h}{nc2}{half}")
                    nc.tensor.matmul(s_ps,
                                     lhsT=st["qt"][h][:, ts(nc2, 128)],
                                     rhs=st["kt"][h][:, ts(half, 512)],
                                     start=True, stop=True)
                    se = statsp.tile([128, 1], FP, tag="sumexp",
                                     name=f"se{b}{h}{nc2}{half}")
                    nc.scalar.activation(out=e_t[:, ts(half, 512)], in_=s_ps,
                                         func=AF.Exp, scale=SCALE, accum_out=se)
                    ses.append(se)
                recip = statsp.tile([128, 1], FP, tag="recip",
                                    name=f"rc{b}{h}{nc2}")
                nc.vector.tensor_add(out=recip, in0=ses[0], in1=ses[1])
                nc.vector.reciprocal(out=recip, in_=recip)
                comb = statsp.tile([128, 1], FP, tag="comb", name=f"cb{b}{h}{nc2}")
                nc.vector.tensor_mul(out=comb, in0=recip,
                                     in1=st["dmask"][:, ds(nc2, 1)])
                # attn = (E * comb) * pmask  (post-softmax masking, no renorm)
                nc.vector.scalar_tensor_tensor(out=e_t, in0=e_t, scalar=comb,
                                               in1=st["pmask"],
                                               op0=OP.mult, op1=OP.mult)
                # bf16 copy for the PV matmul on the idle gpsimd engine
                # (fp32 e_t is what gets stored, keeping attn full precision)
                a_bf = attnbp.tile([128, T], DT, tag="abf", name=f"ab{b}{h}{nc2}")
                nc.gpsimd.tensor_copy(out=a_bf, in_=e_t)
                pt = attnb3p.tile([128, TCH, 128], DT, tag="pt",
                                  name=f"pt{b}{h}{nc2}")
                nc.sync.dma_start_transpose(out=pt, in_=a_bf)
                return e_t, pt

            def pv_unit(b, h, nc2, pt):
                st = state[b]
                o_ps = ps_attn.tile([96, 128], FP, tag="s", name=f"ops{b}{h}{nc2}")
                for tc_i in range(TCH):
                    nc.tensor.matmul(o_ps,
                                     lhsT=st["vt"][tc_i][:, ds(DH * h, DH)],
                                     rhs=pt[:, tc_i, :],
                                     start=(tc_i == 0), stop=(tc_i == TCH - 1))
                t_ = otp.tile([96, 128], DT, tag="ot", name=f"ot{b}{h}{nc2}")
                nc.scalar.activation(out=t_, in_=o_ps, func=AF.Copy)
                st["ot"][(h, nc2)] = t_

            def fin_phase(b):
                st = state[b]
                for nc2 in range(NCH):
                    x = smallp.tile([128, D], FP, tag="x", name=f"x{b}{nc2}")
                    fin = ps_proj.tile([128, D], FP, tag="proj",
                                       name=f"fin{b}{nc2}")
                    for h in range(H):
                        for n0, nn in ((0, 512), (512, 256)):
                            nc.tensor.matmul(fin[:, ds(n0, nn)],
                                             lhsT=st["ot"][(h, nc2)],
                                             rhs=wo_sb[:, h, ds(n0, nn)],
                                             start=(h == 0), stop=(h == H - 1))
                    nc.vector.scalar_tensor_tensor(out=x, in0=fin, scalar=0.0,
                                                   in1=st["drug_x"][:, nc2, :],
                                                   op0=OP.bypass, op1=OP.add)
                    nc.vector.tensor_add(out=x, in0=x, in1=bo_b)
                    stats = statsp.tile([128, 3, 6], FP, tag="bnstats",
                                        name=f"bns{b}{nc2}")
                    xg = x.rearrange("p (g d) -> p g d", g=3)
                    for g in range(3):
                        nc.vector.bn_stats(out=stats[:, g, :], in_=xg[:, g, :])
                    mv = statsp.tile([128, 2], FP, tag="mv", name=f"mv{b}{nc2}")
                    nc.vector.bn_aggr(out=mv, in_=stats)
                    std = statsp.tile([128, 1], FP, tag="std", name=f"sd{b}{nc2}")
                    nc.scalar.activation(out=std, in_=mv[:, ds(1, 1)],
                                         func=AF.Sqrt, bias=eps_t, scale=1.0)
                    rstd = statsp.tile([128, 1], FP, tag="rstd",
                                       name=f"rs{b}{nc2}")
                    nc.vector.reciprocal(out=rstd, in_=std)
                    nc.vector.tensor_scalar(out=x, in0=x,
                                            scalar1=mv[:, ds(0, 1)], scalar2=rstd,
                                            op0=OP.subtract, op1=OP.mult)
                    nc.vector.tensor_mul(out=x, in0=x, in1=gamma_b)
                    nc.vector.tensor_add(out=x, in0=x, in1=beta_b)
                    nc.sync.dma_start(out=out_o[b, ts(nc2, 128), :], in_=x)

            # ---- emission ----
            for item in proj_items(0):
                item()
            for b in range(BL):
                nxt = iter(proj_items(b + 1)) if b + 1 < BL else iter(())
                units = [(h, nc2) for h in range(H) for nc2 in range(NCH)]
                pts = {}
                pending_stores = []

                def flush_stores():
                    for (hh, nn2, et) in pending_stores:
                        nc.sync.dma_start(out=attn_o[b, hh, ts(nn2, 128), :],
                                          in_=et)
                    pending_stores.clear()

                for i, (h, nc2) in enumerate(units):
                    e_t, pt = softmax_unit(b, h, nc2,
                                           alt=(b == BL - 1 and i % 2 == 1))
                    pts[i] = pt
                    pending_stores.append((h, nc2, e_t))
                    if len(pending_stores) >= 2:
                        flush_stores()
                    if i >= PVLAG:
                        hh, nn2 = units[i - PVLAG]
                        pv_unit(b, hh, nn2, pts.pop(i - PVLAG))
                    # interleave up to 3 next-batch projection items per unit
                    for _ in range(3):
                        item = next(nxt, None)
                        if item is None:
                            break
                        item()
                flush_stores()
                for item in nxt:
                    item()
                for i in range(len(units) - PVLAG, len(units)):
                    hh, nn2 = units[i]
                    pv_unit(b, hh, nn2, pts.pop(i))
                fin_phase(b)

    nc.compile()
    return nc


def get_compiled():
    if "nc" not in _compiled:
        _compiled["nc"] = build_bass()
    return _compiled["nc"]


def prep_inputs(drug, target, drug_mask, pro_mask, Wq, Wk, Wv, Wo, bo, gamma, beta):
    """Host-side prep: dtype casts + per-core sharding. Returns in_maps list."""
    bf = ml_dtypes.bfloat16
    drug = np.asarray(drug)
    target = np.asarray(target)
    drug_bf = drug.astype(bf)
    target_bf = target.astype(bf)
    dmask_f = (np.asarray(drug_mask) != 0).astype(np.float32)  # [B, N]
    pmask_f = (np.asarray(pro_mask) != 0).astype(np.float32)   # [B, T]
    # dmask laid out [B, 128, NCH]: dmask[b, p, c] = mask[b, c*128 + p]
    dmask_l = dmask_f.reshape(B, NCH, 128).transpose(0, 2, 1).copy()
    wq_bf = np.asarray(Wq).astype(bf)
    wk_bf = np.asarray(Wk).astype(bf)
    wv_bf = np.asarray(Wv).astype(bf)
    wo_bf = np.asarray(Wo).astype(bf)
    bo_f = np.asarray(bo, dtype=np.float32).astype(bf)
    gamma_f = np.asarray(gamma, dtype=np.float32).astype(bf)
    beta_f = np.asarray(beta, dtype=np.float32).astype(bf)

    in_maps = []
    for c in range(NCORES):
        sl = slice(c * BL, (c + 1) * BL)
        in_maps.append({
            "drug_bf": np.ascontiguousarray(drug_bf[sl]),
            "drug_f": np.ascontiguousarray(drug[sl].astype(np.float32)),
            "target_bf": np.ascontiguousarray(target_bf[sl]),
            "wq": wq_bf, "wk": wk_bf, "wv": wv_bf, "wo": wo_bf,
            "dmask": np.ascontiguousarray(dmask_l[sl]),
            "pmask": np.ascontiguousarray(pmask_f[sl]),
            "bo": bo_f, "gamma": gamma_f, "beta": beta_f,
        })
    return in_maps


def run_on_hw(in_maps, trace=False):
    from concourse.bass_utils import run_bass_kernel_spmd
    nc = get_compiled()
    res = run_bass_kernel_spmd(nc, in_maps, list(range(NCORES)), trace=trace)
    return res


def kernel(**inputs):
    in_maps = prep_inputs(**inputs)
    res = run_on_hw(in_maps)
    outs = [res.results[c]["out_o"] for c in range(NCORES)]
    attns = [res.results[c]["attn_o"] for c in range(NCORES)]
    out_full = np.concatenate(outs, axis=0).astype(np.float32)
    attn_full = np.concatenate(attns, axis=0).astype(np.float32)
    return out_full, attn_full


# revision 35
# speedup vs baseline: 131.0741x; 131.0741x over previous
"""Trainium2 Bass kernel for nn_CrossAttention (cross-attention + post-softmax
masking + residual + LayerNorm), data-parallel over batch across 8 NeuronCores.

Hardcoded problem shapes:
  B=16, N=256 (drug seq), T=1024 (target seq), DRUG_DIM=768, TGT_DIM=2560,
  H=8 heads, DH=96, INNER=768.

Sharding: batch 16 -> 8 cores x 2 local batches. No collectives.

Per-core dataflow (local batch b in {0,1}):
  targetT [2560,1024] and drugT [768,256] loaded via xbar DMA-transpose (bf16)
  QT_h[dh,n]  = Wq_h^T drugT       (per-head, psum fp32 -> sbuf bf16)
  KT_h[dh,t]  = Wk_h^T targetT
  V_tc[t,i]   = targetT_tc^T Wv    (per t-chunk of 128)
  S[n,t]      = QT_h^T KT_h        (psum fp32, per (h, n-chunk of 128))
  E           = exp(S*scale)       (ACT, accum_out = row sums; no max-subtract:
                                    |S*scale| <= ~3 for this data distribution)
  attn        = E * (dmask/rowsum)[per-row] * pmask[per-col]   (post-softmax
                                    masking, no renorm; in-place fp32)
  attn -> DRAM (fp32), attn -> bf16 (ACT cast) -> xbar transpose -> pT[t,n]
  OT_h[dh,n]  = V_h^T pT           (accumulated over t-chunks)
  fin[n,d]    = sum_h OT_h^T Wo_h  (psum fp32)
  out         = LayerNorm(fin + bo + drug) * gamma + beta -> DRAM (fp32)
"""

import numpy as np
import ml_dtypes

B, N, T = 16, 256, 1024
D, TD, H, DH = 768, 2560, 8, 96
INNER = H * DH
EPS = 1e-5
SCALE = DH ** -0.5

NCORES = 8
BL = B // NCORES          # local batches per core
DCH = TD // 128           # 20 contraction chunks for k/v projections
ICH = D // 128            # 6 contraction chunks for q projection
TCH = T // 128            # 8 t-chunks
NCH = N // 128            # 2 n-chunks

BF16 = "bfloat16"
F32 = "float32"

_compiled = {}


def build_bass():
    """Build and compile the per-core Bass program (same program on all 8 cores)."""
    import concourse.bacc as bacc
    import concourse.tile as tile
    import concourse.bass as bass
    from concourse import mybir
    from concourse.bass import ts, ds

    DT = mybir.dt.bfloat16
    FP = mybir.dt.float32
    AF = mybir.ActivationFunctionType
    OP = mybir.AluOpType

    nc = bacc.Bacc("TRN2", target_bir_lowering=False, debug=False,
                   num_devices=NCORES)

    # ---- DRAM I/O ----
    drug_bf = nc.dram_tensor("drug_bf", [BL, N, D], DT, kind="ExternalInput").ap()
    drug_f = nc.dram_tensor("drug_f", [BL, N, D], FP, kind="ExternalInput").ap()
    target_bf = nc.dram_tensor("target_bf", [BL, T, TD], DT, kind="ExternalInput").ap()
    wq_e = nc.dram_tensor("wq", [D, INNER], DT, kind="ExternalInput").ap()
    wk_e = nc.dram_tensor("wk", [TD, INNER], DT, kind="ExternalInput").ap()
    wv_e = nc.dram_tensor("wv", [TD, INNER], DT, kind="ExternalInput").ap()
    wo_e = nc.dram_tensor("wo", [INNER, D], DT, kind="ExternalInput").ap()
    dmask_e = nc.dram_tensor("dmask", [BL, 128, NCH], FP, kind="ExternalInput").ap()
    pmask_e = nc.dram_tensor("pmask", [BL, T], FP, kind="ExternalInput").ap()
    bo_e = nc.dram_tensor("bo", [D], DT, kind="ExternalInput").ap()
    gamma_e = nc.dram_tensor("gamma", [D], DT, kind="ExternalInput").ap()
    beta_e = nc.dram_tensor("beta", [D], DT, kind="ExternalInput").ap()

    attn_o = nc.dram_tensor("attn_o", [BL, H, N, T], FP, kind="ExternalOutput").ap()
    out_o = nc.dram_tensor("out_o", [BL, N, D], FP, kind="ExternalOutput").ap()

    def bcast(ap, p=128):
        return bass.AP(tensor=ap.tensor, offset=ap.offset,
                       ap=[[0, p]] + [list(x) for x in ap.ap])

    with tile.TileContext(nc) as tc:
        with (
            tc.tile_pool(name="singles", bufs=1) as singles,
            tc.tile_pool(name="ttp", bufs=3) as ttp,
            tc.tile_pool(name="proj", bufs=9) as projp,
            tc.tile_pool(name="ktpool", bufs=10) as ktp,
            tc.tile_pool(name="vtpool", bufs=9) as vtp,
            tc.tile_pool(name="small", bufs=2) as smallp,
            tc.tile_pool(name="small1", bufs=1) as small1p,
            tc.tile_pool(name="attnb", bufs=2) as attnbp,
            tc.tile_pool(name="attnb4", bufs=3) as attnb4p,
            tc.tile_pool(name="attnb3", bufs=3) as attnb3p,
            tc.tile_pool(name="attnbE", bufs=3) as attnbEp,
            tc.tile_pool(name="otp", bufs=18) as otp,
            tc.tile_pool(name="stats", bufs=8) as statsp,
            tc.tile_pool(name="ps_proj", bufs=1, space="PSUM") as ps_proj,
            tc.tile_pool(name="ps_attn", bufs=4, space="PSUM") as ps_attn,
            tc.tile_pool(name="ps_ktq", bufs=2, space="PSUM") as ps_ktq,
        ):
            # ---- resident weights / constants ----
            wq_sb = singles.tile([128, ICH, INNER], DT, tag="wq")
            nc.scalar.dma_start(out=wq_sb, in_=wq_e.rearrange("(c p) n -> p c n", p=128))
            wk_sb = singles.tile([128, DCH, INNER], DT, tag="wk")
            wk_r = wk_e.rearrange("(c p) n -> p c n", p=128)
            nc.scalar.dma_start(out=wk_sb[:, 0:10, :], in_=wk_r[:, 0:10, :])
            nc.sync.dma_start(out=wk_sb[:, 10:DCH, :], in_=wk_r[:, 10:DCH, :])
            wv_sb = singles.tile([128, DCH, INNER], DT, tag="wv")
            wo_sb = singles.tile([96, H, D], DT, tag="wo")
            gamma_b = singles.tile([128, D], DT, tag="gamma")
            beta_b = singles.tile([128, D], DT, tag="beta")
            bo_b = singles.tile([128, D], DT, tag="bo")
            eps_t = singles.tile([128, 1], FP, tag="eps")
            nc.vector.memset(eps_t, EPS)

            def load_wv():
                wv_r = wv_e.rearrange("(c p) n -> p c n", p=128)
                nc.scalar.dma_start(out=wv_sb[:, 0:10, :], in_=wv_r[:, 0:10, :])
                nc.sync.dma_start(out=wv_sb[:, 10:DCH, :], in_=wv_r[:, 10:DCH, :])

            def load_tail_weights():
                nc.scalar.dma_start(out=wo_sb,
                                    in_=wo_e.rearrange("(h p) n -> p h n", p=96))
                nc.scalar.dma_start(out=gamma_b, in_=bcast(gamma_e))
                nc.scalar.dma_start(out=beta_b, in_=bcast(beta_e))
                nc.scalar.dma_start(out=bo_b, in_=bcast(bo_e))

            PVLAG = 4  # PV consumption lags softmax emission by this many units

            # Per-batch state built by the projection work items
            state = [dict(qt=[None] * H, kt=None, vt=[None] * TCH, ot={},
                          drugT=None, drug_x=None, pmask=None, dmask=None)
                     for _ in range(BL)]

            def proj_items(b):
                """Work items (closures) for batch b's loads + projections.

                Emitted either directly (first batch) or interleaved into the
                previous batch's attention to keep the in-order PE stream fed.
                PSUM->SBUF cast copies ride on ACT so the DVE stream (softmax)
                never blocks on PE projection progress.
                """
                st = state[b]

                def loads():
                    st["drugT"] = small1p.tile([128, ICH, N], DT, tag="drugT",
                                               name=f"drugT{b}")
                    nc.sync.dma_start_transpose(out=st["drugT"], in_=drug_bf[b])
                    st["drug_x"] = small1p.tile([128, NCH, D], FP, tag="drugx",
                                                name=f"drugx{b}")
                    nc.sync.dma_start(
                        out=st["drug_x"],
                        in_=drug_f[b].rearrange("(c p) d -> p c d", p=128))
                    st["pmask"] = small1p.tile([128, T], FP, tag="pmask",
                                               name=f"pmask{b}")
                    nc.sync.dma_start(out=st["pmask"], in_=bcast(pmask_e[b]))
                    st["dmask"] = smallp.tile([128, NCH], FP, tag="dmask",
                                              name=f"dmask{b}")
                    nc.sync.dma_start(out=st["dmask"], in_=dmask_e[b])
                    st["kt"] = [ktp.tile([96, T], DT, tag="kt",
                                         name=f"kt{b}_{h}") for h in range(H)]
                yield loads

                def qt_item(h):
                    def f():
                        ps = ps_ktq.tile([96, N], FP, tag="ktq", name=f"qtps{b}{h}")
                        for dc in range(ICH):
                            nc.tensor.matmul(ps,
                                             lhsT=wq_sb[:, dc, ds(DH * h, DH)],
                                             rhs=st["drugT"][:, dc, :],
                                             start=(dc == 0), stop=(dc == ICH - 1))
                        t_ = projp.tile([96, N], DT, tag="qt", name=f"qt{b}{h}")
                        nc.scalar.activation(out=t_, in_=ps, func=AF.Copy)
                        st["qt"][h] = t_
                    return f

                tts = {}

                def tt_item(tq):
                    def f():
                        tt = ttp.tile([128, DCH, 256], DT, tag="tt",
                                      name=f"tt{b}{tq}")
                        nc.sync.dma_start_transpose(
                            out=tt, in_=target_bf[b, ts(tq, 256), :])
                        tts[tq] = tt
                    return f

                def kt_item(h, tq):
                    def f():
                        ps = ps_ktq.tile([96, 256], FP, tag="ktq",
                                          name=f"ktps{b}{h}{tq}")
                        for dc in range(DCH):
                            nc.tensor.matmul(ps,
                                             lhsT=wk_sb[:, dc, ds(DH * h, DH)],
                                             rhs=tts[tq][:, dc, :],
                                             start=(dc == 0), stop=(dc == DCH - 1))
                        nc.scalar.activation(out=st["kt"][h][:, ts(tq, 256)],
                                             in_=ps, func=AF.Copy)
                    return f

                def v_item(tq, lc):
                    def f():
                        tc_i = tq * 2 + lc
                        t_ = vtp.tile([128, INNER], DT, tag="vt",
                                      name=f"vt{b}{tc_i}")
                        ps = ps_proj.tile([128, INNER], FP, tag="proj",
                                          name=f"vps{b}{tc_i}")
                        for n0, nn in ((0, 512), (512, 256)):
                            for dc in range(DCH):
                                nc.tensor.matmul(ps[:, ds(n0, nn)],
                                                 lhsT=tts[tq][:, dc, ts(lc, 128)],
                                                 rhs=wv_sb[:, dc, ds(n0, nn)],
                                                 start=(dc == 0),
                                                 stop=(dc == DCH - 1))
                        nc.scalar.activation(out=t_, in_=ps, func=AF.Copy)
                        st["vt"][tc_i] = t_
                    return f

                yield tt_item(0)
                for h in range(H):
                    yield qt_item(h)
                for tq in range(4):
                    if tq + 1 < 4:
                        yield tt_item(tq + 1)
                    if b == 0 and tq == 0:
                        yield load_wv
                    for h in range(H):
                        yield kt_item(h, tq)
                    if b == 0 and tq == 1:
                        yield load_tail_weights
                    yield v_item(tq, 0)
                    yield v_item(tq, 1)

            def softmax_unit(b, h, nc2, alt=False):
                st = state[b]
                ses = []
                spool = ps_ktq if alt else ps_attn
                stag = "ktq" if alt else "s"
                e_t = attnbEp.tile([128, T], FP, tag="E", name=f"E{b}{h}{nc2}")
                for half in range(2):
                    s_ps = spool.tile([128, 512], FP, tag=stag,
                                      name=f"sps{b}{h}{nc2}{half}")
                    nc.tensor.matmul(s_ps,
                                     lhsT=st["qt"][h][:, ts(nc2, 128)],
                                     rhs=st["kt"][h][:, ts(half, 512)],
                                     start=True, stop=True)
                    se = statsp.tile([128, 1], FP, tag="sumexp",
                                     name=f"se{b}{h}{nc2}{half}")
                    nc.scalar.activation(out=e_t[:, ts(half, 512)], in_=s_ps,
                                         func=AF.Exp, scale=SCALE, accum_out=se)
                    ses.append(se)
                recip = statsp.tile([128, 1], FP, tag="recip",
                                    name=f"rc{b}{h}{nc2}")
                nc.vector.tensor_add(out=recip, in0=ses[0], in1=ses[1])
                nc.vector.reciprocal(out=recip, in_=recip)
                comb = statsp.tile([128, 1], FP, tag="comb", name=f"cb{b}{h}{nc2}")
                nc.vector.tensor_mul(out=comb, in0=recip,
                                     in1=st["dmask"][:, ds(nc2, 1)])
                # attn = (E * comb) * pmask  (post-softmax masking, no renorm)
                nc.vector.scalar_tensor_tensor(out=e_t, in0=e_t, scalar=comb,
                                               in1=st["pmask"],
                                               op0=OP.mult, op1=OP.mult)
                # bf16 copy for the PV matmul on the idle gpsimd engine
                # (fp32 e_t is what gets stored, keeping attn full precision)
                a_bf = attnbp.tile([128, T], DT, tag="abf", name=f"ab{b}{h}{nc2}")
                nc.gpsimd.tensor_copy(out=a_bf, in_=e_t)
                pt = attnb3p.tile([128, TCH, 128], DT, tag="pt",
                                  name=f"pt{b}{h}{nc2}")
                nc.sync.dma_start_transpose(out=pt, in_=a_bf)
                return e_t, pt

            def pv_unit(b, h, nc2, pt):
                st = state[b]
                o_ps = ps_attn.tile([96, 128], FP, tag="s", name=f"ops{b}{h}{nc2}")
                for tc_i in range(TCH):
                    nc.tensor.matmul(o_ps,
                                     lhsT=st["vt"][tc_i][:, ds(DH * h, DH)],
                                     rhs=pt[:, tc_i, :],
                                     start=(tc_i == 0), stop=(tc_i == TCH - 1))
                t_ = otp.tile([96, 128], DT, tag="ot", name=f"ot{b}{h}{nc2}")
                nc.scalar.activation(out=t_, in_=o_ps, func=AF.Copy)
                st["ot"][(h, nc2)] = t_

            def fin_phase(b):
                st = state[b]
                for nc2 in range(NCH):
                    x = smallp.tile([128, D], FP, tag="x", name=f"x{b}{nc2}")
                    fin = ps_proj.tile([128, D], FP, tag="proj",
                                       name=f"fin{b}{nc2}")
                    for h in range(H):
                        for n0, nn in ((0, 512), (512, 256)):
                            nc.tensor.matmul(fin[:, ds(n0, nn)],
                                             lhsT=st["ot"][(h, nc2)],
                                             rhs=wo_sb[:, h, ds(n0, nn)],
                                             start=(h == 0), stop=(h == H - 1))
                    nc.vector.scalar_tensor_tensor(out=x, in0=fin, scalar=0.0,
                                                   in1=st["drug_x"][:, nc2, :],
                                                   op0=OP.bypass, op1=OP.add)
                    nc.vector.tensor_add(out=x, in0=x, in1=bo_b)
                    stats = statsp.tile([128, 3, 6], FP, tag="bnstats",
                                        name=f"bns{b}{nc2}")
                    xg = x.rearrange("p (g d) -> p g d", g=3)
                    for g in range(3):
                        nc.vector.bn_stats(out=stats[:, g, :], in_=xg[:, g, :])
                    mv = statsp.tile([128, 2], FP, tag="mv", name=f"mv{b}{nc2}")
                    nc.vector.bn_aggr(out=mv, in_=stats)
                    std = statsp.tile([128, 1], FP, tag="std", name=f"sd{b}{nc2}")
                    nc.scalar.activation(out=std, in_=mv[:, ds(1, 1)],
                                         func=AF.Sqrt, bias=eps_t, scale=1.0)
                    rstd = statsp.tile([128, 1], FP, tag="rstd",
                                       name=f"rs{b}{nc2}")
                    nc.vector.reciprocal(out=rstd, in_=std)
                    nc.vector.tensor_scalar(out=x, in0=x,
                                            scalar1=mv[:, ds(0, 1)], scalar2=rstd,
                                            op0=OP.subtract, op1=OP.mult)
                    nc.vector.tensor_mul(out=x, in0=x, in1=gamma_b)
                    nc.vector.tensor_add(out=x, in0=x, in1=beta_b)
                    nc.sync.dma_start(out=out_o[b, ts(nc2, 128), :], in_=x)

            # ---- emission ----
            for item in proj_items(0):
                item()
            for b in range(BL):
                nxt = iter(proj_items(b + 1)) if b + 1 < BL else iter(())
                units = [(h, nc2) for h in range(H) for nc2 in range(NCH)]
                pts = {}
                pending_stores = []

                def flush_stores():
                    for (hh, nn2, et) in pending_stores:
                        nc.sync.dma_start(out=attn_o[b, hh, ts(nn2, 128), :],
                                          in_=et)
                    pending_stores.clear()

                for i, (h, nc2) in enumerate(units):
                    e_t, pt = softmax_unit(b, h, nc2,
                                           alt=(b == BL - 1 and i % 2 == 1))
                    pts[i] = pt
                    pending_stores.append((h, nc2, e_t))
                    if len(pending_stores) >= 2:
                        flush_stores()
                    if i >= PVLAG:
                        hh, nn2 = units[i - PVLAG]
                        pv_unit(b, hh, nn2, pts.pop(i - PVLAG))
                    # interleave up to 3 next-batch projection items per unit
                    for _ in range(2 if i < 8 else 4):
                        item = next(nxt, None)
                        if item is None:
                            break
                        item()
                flush_stores()
                for item in nxt:
                    item()
                for i in range(len(units) - PVLAG, len(units)):
                    hh, nn2 = units[i]
                    pv_unit(b, hh, nn2, pts.pop(i))
                fin_phase(b)

    nc.compile()
    return nc


def get_compiled():
    if "nc" not in _compiled:
        _compiled["nc"] = build_bass()
    return _compiled["nc"]


def prep_inputs(drug, target, drug_mask, pro_mask, Wq, Wk, Wv, Wo, bo, gamma, beta):
    """Host-side prep: dtype casts + per-core sharding. Returns in_maps list."""
    bf = ml_dtypes.bfloat16
    drug = np.asarray(drug)
    target = np.asarray(target)
    drug_bf = drug.astype(bf)
    target_bf = target.astype(bf)
    dmask_f = (np.asarray(drug_mask) != 0).astype(np.float32)  # [B, N]
    pmask_f = (np.asarray(pro_mask) != 0).astype(np.float32)   # [B, T]
    # dmask laid out [B, 128, NCH]: dmask[b, p, c] = mask[b, c*128 + p]
    dmask_l = dmask_f.reshape(B, NCH, 128).transpose(0, 2, 1).copy()
    wq_bf = np.asarray(Wq).astype(bf)
    wk_bf = np.asarray(Wk).astype(bf)
    wv_bf = np.asarray(Wv).astype(bf)
    wo_bf = np.asarray(Wo).astype(bf)
    bo_f = np.asarray(bo, dtype=np.float32).astype(bf)
    gamma_f = np.asarray(gamma, dtype=np.float32).astype(bf)
    beta_f = np.asarray(beta, dtype=np.float32).astype(bf)

    in_maps = []
    for c in range(NCORES):
        sl = slice(c * BL, (c + 1) * BL)
        in_maps.append({
            "drug_bf": np.ascontiguousarray(drug_bf[sl]),
            "drug_f": np.ascontiguousarray(drug[sl].astype(np.float32)),
            "target_bf": np.ascontiguousarray(target_bf[sl]),
            "wq": wq_bf, "wk": wk_bf, "wv": wv_bf, "wo": wo_bf,
            "dmask": np.ascontiguousarray(dmask_l[sl]),
            "pmask": np.ascontiguousarray(pmask_f[sl]),
            "bo": bo_f, "gamma": gamma_f, "beta": beta_f,
        })
    return in_maps


def run_on_hw(in_maps, trace=False):
    from concourse.bass_utils import run_bass_kernel_spmd
    nc = get_compiled()
    res = run_bass_kernel_spmd(nc, in_maps, list(range(NCORES)), trace=trace)
    return res


def kernel(**inputs):
    in_maps = prep_inputs(**inputs)
    res = run_on_hw(in_maps)
    outs = [res.results[c]["out_o"] for c in range(NCORES)]
    attns = [res.results[c]["attn_o"] for c in range(NCORES)]
    out_full = np.concatenate(outs, axis=0).astype(np.float32)
    attn_full = np.concatenate(attns, axis=0).astype(np.float32)
    return out_full, attn_full


# revision 42
# speedup vs baseline: 133.4398x; 1.0180x over previous
"""Trainium2 Bass kernel for nn_CrossAttention (cross-attention + post-softmax
masking + residual + LayerNorm), data-parallel over batch across 8 NeuronCores.

Hardcoded problem shapes:
  B=16, N=256 (drug seq), T=1024 (target seq), DRUG_DIM=768, TGT_DIM=2560,
  H=8 heads, DH=96, INNER=768.

Sharding: batch 16 -> 8 cores x 2 local batches. No collectives.

Per-core dataflow (local batch b in {0,1}):
  targetT [2560,1024] and drugT [768,256] loaded via xbar DMA-transpose (bf16)
  QT_h[dh,n]  = Wq_h^T drugT       (per-head, psum fp32 -> sbuf bf16)
  KT_h[dh,t]  = Wk_h^T targetT
  V_tc[t,i]   = targetT_tc^T Wv    (per t-chunk of 128)
  S[n,t]      = QT_h^T KT_h        (psum fp32, per (h, n-chunk of 128))
  E           = exp(S*scale)       (ACT, accum_out = row sums; no max-subtract:
                                    |S*scale| <= ~3 for this data distribution)
  attn        = E * (dmask/rowsum)[per-row] * pmask[per-col]   (post-softmax
                                    masking, no renorm; in-place fp32)
  attn -> DRAM (fp32), attn -> bf16 (ACT cast) -> xbar transpose -> pT[t,n]
  OT_h[dh,n]  = V_h^T pT           (accumulated over t-chunks)
  fin[n,d]    = sum_h OT_h^T Wo_h  (psum fp32)
  out         = LayerNorm(fin + bo + drug) * gamma + beta -> DRAM (fp32)
"""

import numpy as np
import ml_dtypes

B, N, T = 16, 256, 1024
D, TD, H, DH = 768, 2560, 8, 96
INNER = H * DH
EPS = 1e-5
SCALE = DH ** -0.5

NCORES = 8
BL = B // NCORES          # local batches per core
DCH = TD // 128           # 20 contraction chunks for k/v projections
ICH = D // 128            # 6 contraction chunks for q projection
TCH = T // 128            # 8 t-chunks
NCH = N // 128            # 2 n-chunks

BF16 = "bfloat16"
F32 = "float32"

_compiled = {}


def build_bass():
    """Build and compile the per-core Bass program (same program on all 8 cores)."""
    import concourse.bacc as bacc
    import concourse.tile as tile
    import concourse.bass as bass
    from concourse import mybir
    from concourse.bass import ts, ds

    DT = mybir.dt.bfloat16
    FP = mybir.dt.float32
    AF = mybir.ActivationFunctionType
    OP = mybir.AluOpType

    nc = bacc.Bacc("TRN2", target_bir_lowering=False, debug=False,
                   num_devices=NCORES)

    # ---- DRAM I/O ----
    drug_bf = nc.dram_tensor("drug_bf", [BL, N, D], DT, kind="ExternalInput").ap()
    drug_f = nc.dram_tensor("drug_f", [BL, N, D], FP, kind="ExternalInput").ap()
    target_bf = nc.dram_tensor("target_bf", [BL, T, TD], DT, kind="ExternalInput").ap()
    wq_e = nc.dram_tensor("wq", [D, INNER], DT, kind="ExternalInput").ap()
    wk_e = nc.dram_tensor("wk", [TD, INNER], DT, kind="ExternalInput").ap()
    wv_e = nc.dram_tensor("wv", [TD, INNER], DT, kind="ExternalInput").ap()
    wo_e = nc.dram_tensor("wo", [INNER, D], DT, kind="ExternalInput").ap()
    dmask_e = nc.dram_tensor("dmask", [BL, 128, NCH], FP, kind="ExternalInput").ap()
    pmask_e = nc.dram_tensor("pmask", [BL, T], FP, kind="ExternalInput").ap()
    bo_e = nc.dram_tensor("bo", [D], DT, kind="ExternalInput").ap()
    gamma_e = nc.dram_tensor("gamma", [D], DT, kind="ExternalInput").ap()
    beta_e = nc.dram_tensor("beta", [D], DT, kind="ExternalInput").ap()

    attn_o = nc.dram_tensor("attn_o", [BL, H, N, T], FP, kind="ExternalOutput").ap()
    out_o = nc.dram_tensor("out_o", [BL, N, D], FP, kind="ExternalOutput").ap()

    def bcast(ap, p=128):
        return bass.AP(tensor=ap.tensor, offset=ap.offset,
                       ap=[[0, p]] + [list(x) for x in ap.ap])

    with tile.TileContext(nc) as tc:
        with (
            tc.tile_pool(name="singles", bufs=1) as singles,
            tc.tile_pool(name="ttp", bufs=3) as ttp,
            tc.tile_pool(name="proj", bufs=9) as projp,
            tc.tile_pool(name="ktpool", bufs=10) as ktp,
            tc.tile_pool(name="vtpool", bufs=9) as vtp,
            tc.tile_pool(name="small", bufs=2) as smallp,
            tc.tile_pool(name="small1", bufs=1) as small1p,
            tc.tile_pool(name="attnb", bufs=2) as attnbp,
            tc.tile_pool(name="attnb4", bufs=3) as attnb4p,
            tc.tile_pool(name="attnb3", bufs=4) as attnb3p,
            tc.tile_pool(name="attnbE", bufs=3) as attnbEp,
            tc.tile_pool(name="otp", bufs=18) as otp,
            tc.tile_pool(name="stats", bufs=8) as statsp,
            tc.tile_pool(name="ps_proj", bufs=1, space="PSUM") as ps_proj,
            tc.tile_pool(name="ps_attn", bufs=4, space="PSUM") as ps_attn,
            tc.tile_pool(name="ps_ktq", bufs=2, space="PSUM") as ps_ktq,
        ):
            # ---- resident weights / constants ----
            wq_sb = singles.tile([128, ICH, INNER], DT, tag="wq")
            nc.scalar.dma_start(out=wq_sb, in_=wq_e.rearrange("(c p) n -> p c n", p=128))
            wk_sb = singles.tile([128, DCH, INNER], DT, tag="wk")
            wk_r = wk_e.rearrange("(c p) n -> p c n", p=128)
            nc.scalar.dma_start(out=wk_sb[:, 0:10, :], in_=wk_r[:, 0:10, :])
            nc.sync.dma_start(out=wk_sb[:, 10:DCH, :], in_=wk_r[:, 10:DCH, :])
            wv_sb = singles.tile([128, DCH, INNER], DT, tag="wv")
            wo_sb = singles.tile([96, H, D], DT, tag="wo")
            gamma_b = singles.tile([128, D], DT, tag="gamma")
            beta_b = singles.tile([128, D], DT, tag="beta")
            bo_b = singles.tile([128, D], DT, tag="bo")
            eps_t = singles.tile([128, 1], FP, tag="eps")
            nc.vector.memset(eps_t, EPS)

            def load_wv():
                wv_r = wv_e.rearrange("(c p) n -> p c n", p=128)
                nc.scalar.dma_start(out=wv_sb[:, 0:10, :], in_=wv_r[:, 0:10, :])
                nc.sync.dma_start(out=wv_sb[:, 10:DCH, :], in_=wv_r[:, 10:DCH, :])

            def load_tail_weights():
                nc.scalar.dma_start(out=wo_sb,
                                    in_=wo_e.rearrange("(h p) n -> p h n", p=96))
                nc.scalar.dma_start(out=gamma_b, in_=bcast(gamma_e))
                nc.scalar.dma_start(out=beta_b, in_=bcast(beta_e))
                nc.scalar.dma_start(out=bo_b, in_=bcast(bo_e))

            PVLAG = 4  # PV consumption lags softmax emission by this many units

            # Per-batch state built by the projection work items
            state = [dict(qt=[None] * H, kt=None, vt=[None] * TCH, ot={},
                          drugT=None, drug_x=None, pmask=None, dmask=None)
                     for _ in range(BL)]

            def proj_items(b):
                """Work items (closures) for batch b's loads + projections.

                Emitted either directly (first batch) or interleaved into the
                previous batch's attention to keep the in-order PE stream fed.
                PSUM->SBUF cast copies ride on ACT so the DVE stream (softmax)
                never blocks on PE projection progress.
                """
                st = state[b]

                def loads():
                    st["drugT"] = small1p.tile([128, ICH, N], DT, tag="drugT",
                                               name=f"drugT{b}")
                    nc.sync.dma_start_transpose(out=st["drugT"], in_=drug_bf[b])
                    st["drug_x"] = small1p.tile([128, NCH, D], FP, tag="drugx",
                                                name=f"drugx{b}")
                    nc.sync.dma_start(
                        out=st["drug_x"],
                        in_=drug_f[b].rearrange("(c p) d -> p c d", p=128))
                    st["pmask"] = small1p.tile([128, T], FP, tag="pmask",
                                               name=f"pmask{b}")
                    nc.sync.dma_start(out=st["pmask"], in_=bcast(pmask_e[b]))
                    st["dmask"] = smallp.tile([128, NCH], FP, tag="dmask",
                                              name=f"dmask{b}")
                    nc.sync.dma_start(out=st["dmask"], in_=dmask_e[b])
                    st["kt"] = [ktp.tile([96, T], DT, tag="kt",
                                         name=f"kt{b}_{h}") for h in range(H)]
                yield loads

                def qt_item(h):
                    def f():
                        ps = ps_ktq.tile([96, N], FP, tag="ktq", name=f"qtps{b}{h}")
                        for dc in range(ICH):
                            nc.tensor.matmul(ps,
                                             lhsT=wq_sb[:, dc, ds(DH * h, DH)],
                                             rhs=st["drugT"][:, dc, :],
                                             start=(dc == 0), stop=(dc == ICH - 1))
                        t_ = projp.tile([96, N], DT, tag="qt", name=f"qt{b}{h}")
                        nc.scalar.activation(out=t_, in_=ps, func=AF.Copy)
                        st["qt"][h] = t_
                    return f

                tts = {}

                def tt_item(tq):
                    def f():
                        tt = ttp.tile([128, DCH, 256], DT, tag="tt",
                                      name=f"tt{b}{tq}")
                        nc.sync.dma_start_transpose(
                            out=tt, in_=target_bf[b, ts(tq, 256), :])
                        tts[tq] = tt
                    return f

                def kt_item(h, tq):
                    def f():
                        ps = ps_ktq.tile([96, 256], FP, tag="ktq",
                                          name=f"ktps{b}{h}{tq}")
                        for dc in range(DCH):
                            nc.tensor.matmul(ps,
                                             lhsT=wk_sb[:, dc, ds(DH * h, DH)],
                                             rhs=tts[tq][:, dc, :],
                                             start=(dc == 0), stop=(dc == DCH - 1))
                        nc.scalar.activation(out=st["kt"][h][:, ts(tq, 256)],
                                             in_=ps, func=AF.Copy)
                    return f

                def v_item(tq, lc):
                    def f():
                        tc_i = tq * 2 + lc
                        t_ = vtp.tile([128, INNER], DT, tag="vt",
                                      name=f"vt{b}{tc_i}")
                        ps = ps_proj.tile([128, INNER], FP, tag="proj",
                                          name=f"vps{b}{tc_i}")
                        for n0, nn in ((0, 512), (512, 256)):
                            for dc in range(DCH):
                                nc.tensor.matmul(ps[:, ds(n0, nn)],
                                                 lhsT=tts[tq][:, dc, ts(lc, 128)],
                                                 rhs=wv_sb[:, dc, ds(n0, nn)],
                                                 start=(dc == 0),
                                                 stop=(dc == DCH - 1))
                        nc.scalar.activation(out=t_, in_=ps, func=AF.Copy)
                        st["vt"][tc_i] = t_
                    return f

                yield tt_item(0)
                for h in range(H):
                    yield qt_item(h)
                for tq in range(4):
                    if tq + 1 < 4:
                        yield tt_item(tq + 1)
                    if b == 0 and tq == 0:
                        yield load_wv
                    for h in range(H):
                        yield kt_item(h, tq)
                    if b == 0 and tq == 1:
                        yield load_tail_weights
                    yield v_item(tq, 0)
                    yield v_item(tq, 1)

            def softmax_unit(b, h, nc2, alt=False):
                st = state[b]
                ses = []
                spool = ps_ktq if alt else ps_attn
                stag = "ktq" if alt else "s"
                e_t = attnbEp.tile([128, T], FP, tag="E", name=f"E{b}{h}{nc2}")
                for half in range(2):
                    s_ps = spool.tile([128, 512], FP, tag=stag,
                                      name=f"sps{b}{h}{nc2}{half}")
                    nc.tensor.matmul(s_ps,
                                     lhsT=st["qt"][h][:, ts(nc2, 128)],
                                     rhs=st["kt"][h][:, ts(half, 512)],
                                     start=True, stop=True)
                    se = statsp.tile([128, 1], FP, tag="sumexp",
                                     name=f"se{b}{h}{nc2}{half}")
                    nc.scalar.activation(out=e_t[:, ts(half, 512)], in_=s_ps,
                                         func=AF.Exp, scale=SCALE, accum_out=se)
                    ses.append(se)
                recip = statsp.tile([128, 1], FP, tag="recip",
                                    name=f"rc{b}{h}{nc2}")
                nc.vector.tensor_add(out=recip, in0=ses[0], in1=ses[1])
                nc.vector.reciprocal(out=recip, in_=recip)
                comb = statsp.tile([128, 1], FP, tag="comb", name=f"cb{b}{h}{nc2}")
                nc.vector.tensor_mul(out=comb, in0=recip,
                                     in1=st["dmask"][:, ds(nc2, 1)])
                # attn = (E * comb) * pmask  (post-softmax masking, no renorm)
                nc.vector.scalar_tensor_tensor(out=e_t, in0=e_t, scalar=comb,
                                               in1=st["pmask"],
                                               op0=OP.mult, op1=OP.mult)
                # bf16 copy for the PV matmul on the idle gpsimd engine
                # (fp32 e_t is what gets stored, keeping attn full precision)
                a_bf = attnbp.tile([128, T], DT, tag="abf", name=f"ab{b}{h}{nc2}")
                nc.gpsimd.tensor_copy(out=a_bf, in_=e_t)
                pt = attnb3p.tile([128, TCH, 128], DT, tag="pt",
                                  name=f"pt{b}{h}{nc2}")
                nc.sync.dma_start_transpose(out=pt, in_=a_bf)
                return e_t, pt

            def pv_unit(b, h, nc2, pt):
                st = state[b]
                o_ps = ps_attn.tile([96, 128], FP, tag="s", name=f"ops{b}{h}{nc2}")
                for tc_i in range(TCH):
                    nc.tensor.matmul(o_ps,
                                     lhsT=st["vt"][tc_i][:, ds(DH * h, DH)],
                                     rhs=pt[:, tc_i, :],
                                     start=(tc_i == 0), stop=(tc_i == TCH - 1))
                t_ = otp.tile([96, 128], DT, tag="ot", name=f"ot{b}{h}{nc2}")
                nc.scalar.activation(out=t_, in_=o_ps, func=AF.Copy)
                st["ot"][(h, nc2)] = t_

            def fin_phase(b):
                st = state[b]
                for nc2 in range(NCH):
                    x = smallp.tile([128, D], FP, tag="x", name=f"x{b}{nc2}")
                    fin = ps_proj.tile([128, D], FP, tag="proj",
                                       name=f"fin{b}{nc2}")
                    for h in range(H):
                        for n0, nn in ((0, 512), (512, 256)):
                            nc.tensor.matmul(fin[:, ds(n0, nn)],
                                             lhsT=st["ot"][(h, nc2)],
                                             rhs=wo_sb[:, h, ds(n0, nn)],
                                             start=(h == 0), stop=(h == H - 1))
                    nc.vector.scalar_tensor_tensor(out=x, in0=fin, scalar=0.0,
                                                   in1=st["drug_x"][:, nc2, :],
                                                   op0=OP.bypass, op1=OP.add)
                    nc.vector.tensor_add(out=x, in0=x, in1=bo_b)
                    stats = statsp.tile([128, 3, 6], FP, tag="bnstats",
                                        name=f"bns{b}{nc2}")
                    xg = x.rearrange("p (g d) -> p g d", g=3)
                    for g in range(3):
                        nc.vector.bn_stats(out=stats[:, g, :], in_=xg[:, g, :])
                    mv = statsp.tile([128, 2], FP, tag="mv", name=f"mv{b}{nc2}")
                    nc.vector.bn_aggr(out=mv, in_=stats)
                    std = statsp.tile([128, 1], FP, tag="std", name=f"sd{b}{nc2}")
                    nc.scalar.activation(out=std, in_=mv[:, ds(1, 1)],
                                         func=AF.Sqrt, bias=eps_t, scale=1.0)
                    rstd = statsp.tile([128, 1], FP, tag="rstd",
                                       name=f"rs{b}{nc2}")
                    nc.vector.reciprocal(out=rstd, in_=std)
                    nc.vector.tensor_scalar(out=x, in0=x,
                                            scalar1=mv[:, ds(0, 1)], scalar2=rstd,
                                            op0=OP.subtract, op1=OP.mult)
                    nc.vector.tensor_mul(out=x, in0=x, in1=gamma_b)
                    nc.vector.tensor_add(out=x, in0=x, in1=beta_b)
                    nc.sync.dma_start(out=out_o[b, ts(nc2, 128), :], in_=x)

            # ---- emission ----
            for item in proj_items(0):
                item()
            for b in range(BL):
                nxt = iter(proj_items(b + 1)) if b + 1 < BL else iter(())
                units = [(h, nc2) for h in range(H) for nc2 in range(NCH)]
                pts = {}
                pending_stores = []

                def flush_stores():
                    for (hh, nn2, et) in pending_stores:
                        nc.sync.dma_start(out=attn_o[b, hh, ts(nn2, 128), :],
                                          in_=et)
                    pending_stores.clear()

                for i, (h, nc2) in enumerate(units):
                    e_t, pt = softmax_unit(b, h, nc2,
                                           alt=(b == BL - 1 and i % 2 == 1))
                    pts[i] = pt
                    pending_stores.append((h, nc2, e_t))
                    if len(pending_stores) >= 2:
                        flush_stores()
                    if i >= PVLAG:
                        hh, nn2 = units[i - PVLAG]
                        pv_unit(b, hh, nn2, pts.pop(i - PVLAG))
                    # interleave up to 3 next-batch projection items per unit
                    for _ in range(2 if i < 8 else 4):
                        item = next(nxt, None)
                        if item is None:
                            break
                        item()
                flush_stores()
                for item in nxt:
                    item()
                for i in range(len(units) - PVLAG, len(units)):
                    hh, nn2 = units[i]
                    pv_unit(b, hh, nn2, pts.pop(i))
                fin_phase(b)

    nc.compile()
    return nc


def get_compiled():
    if "nc" not in _compiled:
        _compiled["nc"] = build_bass()
    return _compiled["nc"]


def prep_inputs(drug, target, drug_mask, pro_mask, Wq, Wk, Wv, Wo, bo, gamma, beta):
    """Host-side prep: dtype casts + per-core sharding. Returns in_maps list."""
    bf = ml_dtypes.bfloat16
    drug = np.asarray(drug)
    target = np.asarray(target)
    drug_bf = drug.astype(bf)
    target_bf = target.astype(bf)
    dmask_f = (np.asarray(drug_mask) != 0).astype(np.float32)  # [B, N]
    pmask_f = (np.asarray(pro_mask) != 0).astype(np.float32)   # [B, T]
    # dmask laid out [B, 128, NCH]: dmask[b, p, c] = mask[b, c*128 + p]
    dmask_l = dmask_f.reshape(B, NCH, 128).transpose(0, 2, 1).copy()
    wq_bf = np.asarray(Wq).astype(bf)
    wk_bf = np.asarray(Wk).astype(bf)
    wv_bf = np.asarray(Wv).astype(bf)
    wo_bf = np.asarray(Wo).astype(bf)
    bo_f = np.asarray(bo, dtype=np.float32).astype(bf)
    gamma_f = np.asarray(gamma, dtype=np.float32).astype(bf)
    beta_f = np.asarray(beta, dtype=np.float32).astype(bf)

    in_maps = []
    for c in range(NCORES):
        sl = slice(c * BL, (c + 1) * BL)
        in_maps.append({
            "drug_bf": np.ascontiguousarray(drug_bf[sl]),
            "drug_f": np.ascontiguousarray(drug[sl].astype(np.float32)),
            "target_bf": np.ascontiguousarray(target_bf[sl]),
            "wq": wq_bf, "wk": wk_bf, "wv": wv_bf, "wo": wo_bf,
            "dmask": np.ascontiguousarray(dmask_l[sl]),
            "pmask": np.ascontiguousarray(pmask_f[sl]),
            "bo": bo_f, "gamma": gamma_f, "beta": beta_f,
        })
    return in_maps


def run_on_hw(in_maps, trace=False):
    from concourse.bass_utils import run_bass_kernel_spmd
    nc = get_compiled()
    res = run_bass_kernel_spmd(nc, in_maps, list(range(NCORES)), trace=trace)
    return res


def kernel(**inputs):
    in_maps = prep_inputs(**inputs)
    res = run_on_hw(in_maps)
    outs = [res.results[c]["out_o"] for c in range(NCORES)]
    attns = [res.results[c]["attn_o"] for c in range(NCORES)]
    out_full = np.concatenate(outs, axis=0).astype(np.float32)
    attn_full = np.concatenate(attns, axis=0).astype(np.float32)
    return out_full, attn_full
